# revision 13
# baseline (speedup 1.0000x reference)
"""v4: 4-layer GCN encoder on 8 Trainium2 NeuronCores.

Changes over v3.1:
  - 3-way source-region split (A=28, B1=11, B2=10 blocks per core). Each
    layer runs three AllGathers: AG-A launches ~55% into the previous
    layer's aggregation, AG-B1 at ~77%, AG-B2 at the boundary. The last
    collective is ~5MB, so it lands before the pipeline's part-2 gathers
    need it -- removing the per-layer h1-wait stalls of the 2-way split.
  - Aggregation pipeline: stage s emits [diag+gather-A](s),
    [mmA, gather-B1, gather-B2](s-1), [mmB1, mmB2, epilogue](s-2).
  - 16KB SWDGE scratch (ring = one 1024-idx call, baseline-proven).
"""

import numpy as np
import ml_dtypes

import concourse.bacc as bacc
import concourse.mybir as mybir
import concourse.tile as tile
from concourse.bass_utils import run_bass_kernel_spmd

P = 128
BF16 = mybir.dt.bfloat16
F32 = mybir.dt.float32
I16 = mybir.dt.int16

NC = 8
NB = 49
NPART = 3
RB = [28, 11, 10]                  # blocks per region (A, B1, B2)
RBASE = [0, 28, 39]                # first block of each region
RSZ = [NC * b * P for b in RB]     # region row counts: 28672, 11264, 10240
ROFF = [0, RSZ[0], RSZ[0] + RSZ[1]]
G = 4
NGRP = (NB + G - 1) // G           # 13 groups (last has 1 block)
NPAD = sum(RSZ)                    # 50176
N_NODES = 50000
N_EDGES = 800000
IN_CH = 512
HID = 256
FC_IN = IN_CH // P                 # 4
FH = HID // P                      # 2
CT = 8                             # max tiles per gather call (1024 idx cap)
# post-group hooks: dense+AG for region r fires after this group's epilogue
POST_GRP = [6, 9, 12]              # blocks 0-27 / 28-39 / 40-48 covered


def group_blocks():
    return [(g, list(range(g * G, min((g + 1) * G, NB)))) for g in range(NGRP)]


def _region_of_block(j):
    if j < 28:
        return 0
    return 1 if j < 39 else 2


# ----------------------------------------------------------------- host prep

def _preprocess(edge_index, edge_weight):
    src0 = np.asarray(edge_index[0], dtype=np.int64)
    dst0 = np.asarray(edge_index[1], dtype=np.int64)
    ew0 = np.asarray(edge_weight, dtype=np.float32)
    N = N_NODES

    deg = np.bincount(dst0, weights=ew0.astype(np.float64), minlength=N)
    deg = (deg + 1.0).astype(np.float32)
    dis = (1.0 / np.sqrt(deg)).astype(np.float32)

    indeg = np.bincount(dst0, minlength=N)
    order = np.argsort(-indeg, kind="stable")
    nbins = NC * NB
    rank = np.empty(N, dtype=np.int64)
    rank[order] = np.arange(N)
    bin_of = rank % nbins
    core_of = bin_of % NC
    slot_of = bin_of // NC
    lane_of = rank // nbins
    assert lane_of.max() < P

    ecore = core_of[dst0]
    eslot = slot_of[dst0]
    cnt_cs = np.zeros((NC, NB), dtype=np.int64)
    np.add.at(cnt_cs, (ecore, eslot), 1)

    # rank slots by count desc; deal round-robin into the 3 regions so each
    # region's per-rank profiles match across cores
    slot2blk = np.zeros((NC, NB), dtype=np.int64)
    deal = []
    ri = [0, 0, 0]
    for k in range(NB):
        r = k % NPART
        # keep dealing into regions that still have space
        while ri[r] >= RB[r]:
            r = (r + 1) % NPART
        deal.append((r, ri[r]))
        ri[r] += 1
    for c in range(NC):
        ranked = np.argsort(-cnt_cs[c], kind="stable")
        for k, s in enumerate(ranked):
            r, pos = deal[k]
            slot2blk[c, s] = RBASE[r] + pos
    blk_of = slot2blk[core_of, slot_of]

    reg_of = np.where(blk_of < 28, 0, np.where(blk_of < 39, 1, 2))
    base = np.zeros(N, dtype=np.int64)
    for r in range(NPART):
        m = reg_of == r
        base[m] = ROFF[r] + (core_of[m] * RB[r] + (blk_of[m] - RBASE[r])) * P
    gpos = base + lane_of

    # edge part = src's region
    h_of = reg_of[src0]
    ej = blk_of[dst0]
    ec = core_of[dst0]
    cnt = np.zeros((NC, NB, NPART), dtype=np.int64)
    np.add.at(cnt, (ec, ej, h_of), 1)
    tiles = np.ceil(cnt.max(axis=0) / P).astype(np.int64)
    tiles = np.maximum(tiles, 1)

    tile_base = np.zeros((NB, NPART), dtype=np.int64)
    call_ranges = []                                # (g, part) -> (t0, t1)
    t = 0
    for g, blks in group_blocks():
        for r in range(NPART):
            t0 = t
            for j in blks:
                tile_base[j, r] = t
                t += tiles[j, r]
            call_ranges.append((g, r, t0, t))
    ntiles = int(t)
    nslots = ntiles * P

    ekey = (ec * NB + ej) * NPART + h_of
    esort = np.lexsort((gpos[src0], ekey))
    ks = ekey[esort]
    uniq, inv, counts = np.unique(ks, return_inverse=True, return_counts=True)
    starts = np.zeros_like(counts)
    starts[1:] = np.cumsum(counts)[:-1]
    rib = np.arange(len(esort)) - starts[inv]

    e_c = ec[esort]
    e_j = ej[esort]
    e_h = h_of[esort]
    e_src = src0[esort]
    e_dst = dst0[esort]
    e_w = ew0[esort]
    q = (tile_base[e_j, e_h] + rib // P) * P + rib % P

    gidx = np.zeros((NC, nslots), dtype=np.int16)
    scmp = np.zeros((NC, P, ntiles), dtype=np.float32)
    sew = np.zeros((NC, P, ntiles), dtype=np.float32)

    roff = np.array(ROFF, dtype=np.int64)
    idxv = gpos[e_src] - roff[e_h]
    assert idxv.max() < 32768
    for c in range(NC):
        m = e_c == c
        qc = q[m]
        gidx[c, qc] = idxv[m].astype(np.int16)
        scmp[c, qc % P, qc // P] = lane_of[e_dst[m]].astype(np.float32)
        sew[c, qc % P, qc // P] = e_w[m] * dis[e_dst[m]]

    def wrap(a):
        w = a.reshape(NC, ntiles * 8, 16).transpose(0, 2, 1)
        return np.ascontiguousarray(np.tile(w, (1, 8, 1)))

    disc = np.zeros((NC, P, NB), dtype=np.float32)
    disc[core_of, lane_of, blk_of] = dis

    eyedis = np.zeros((NC, P, NB * P), dtype=np.float32)
    for c in range(NC):
        m = core_of == c
        eyedis[c, lane_of[m], blk_of[m] * P + lane_of[m]] = dis[m]

    nid = np.full((NC, NB, P), -1, dtype=np.int64)
    nid[core_of, blk_of, lane_of] = np.arange(N)

    return dict(
        tiles=tiles, tile_base=tile_base, ntiles=ntiles, nslots=nslots,
        call_ranges=call_ranges,
        gidx=wrap(gidx),
        scmp=scmp.astype(ml_dtypes.bfloat16), sew=sew.astype(ml_dtypes.bfloat16),
        disc=disc, eyedis=eyedis.astype(ml_dtypes.bfloat16),
        dis=dis, gpos=gpos, nid=nid,
    )


def _pack_xown(x, nid_c):
    """x^T tiles for the core's own 49 blocks, one row-block per j:
    xo[j*P + p, fc*P + lane] = x[node(c, j, lane), fc*P + p]."""
    out = np.zeros((NB * P, FC_IN * P), dtype=np.float32)
    for j in range(NB):
        nods = nid_c[j]
        ok = nods >= 0
        xv = x[nods[ok]]
        for fc in range(FC_IN):
            out[j * P:(j + 1) * P, fc * P:fc * P + P][:, ok] = \
                xv[:, fc * P:(fc + 1) * P].T
    return out.astype(ml_dtypes.bfloat16)


def _pack_wcat(Ws):
    cols = []
    for Wl in Ws:
        k = Wl.shape[0]
        for fc in range(k // P):
            cols.append(Wl[fc * P:(fc + 1) * P, :])
    return np.concatenate(cols, axis=1).astype(ml_dtypes.bfloat16)


def _pack_bias(bs):
    out = np.zeros((P, 8), dtype=np.float32)
    for l, b in enumerate(bs):
        for fh in range(FH):
            out[:, l * 2 + fh] = b[fh * P:(fh + 1) * P]
    return out


# ----------------------------------------------------------------- builder

def _build(prep):
    tiles = prep["tiles"]
    tile_base = prep["tile_base"]
    ntiles = prep["ntiles"]
    call_ranges = prep["call_ranges"]
    maxg = [0] * NPART
    for (g, r, t0, t1) in call_ranges:
        maxg[r] = max(maxg[r], t1 - t0)

    nc = bacc.Bacc("TRN2", target_bir_lowering=False, debug=False,
                   num_devices=NC, num_swdge_queues=4,
                   dynamic_dma_scratch_size=16384)
    qctr = [0]

    gidx_d = nc.dram_tensor("gidx", [P, ntiles * 8], I16, kind="ExternalInput")
    scmp_d = nc.dram_tensor("scmp", [P, ntiles], BF16, kind="ExternalInput")
    sew_d = nc.dram_tensor("sew", [P, ntiles], BF16, kind="ExternalInput")
    iota_d = nc.dram_tensor("iota", [P, P], BF16, kind="ExternalInput")
    eyedis_d = nc.dram_tensor("eyedis", [P, NB * P], BF16, kind="ExternalInput")
    wcat_cols = (FC_IN + 3 * FH) * HID
    wcat_d = nc.dram_tensor("wcat", [P, wcat_cols], BF16, kind="ExternalInput")
    bias_d = nc.dram_tensor("bias", [P, 8], F32, kind="ExternalInput")
    prelu_d = nc.dram_tensor("prelua", [P, 2], F32, kind="ExternalInput")
    disc_d = nc.dram_tensor("disc", [P, NB], F32, kind="ExternalInput")
    xown_d = nc.dram_tensor("xown", [NB * P, FC_IN * P], BF16,
                            kind="ExternalInput")
    out_d = nc.dram_tensor("out", [FH * NB * P, P], F32, kind="ExternalOutput")

    w_off = {}
    off = 0
    for l in range(4):
        k = FC_IN if l == 0 else FH
        for fc in range(k):
            w_off[(l, fc)] = off
            off += HID

    with tile.TileContext(nc) as tc:
        with (
            tc.tile_pool(name="res", bufs=1) as res,
            tc.tile_pool(name="xpool", bufs=4) as xpool,
            tc.tile_pool(name="akeep", bufs=1) as akeep,
            tc.tile_pool(name="mpool", bufs=2) as mpool,
            tc.tile_pool(name="spool", bufs=2) as spool,
            tc.tile_pool(name="htpool", bufs=1) as htpool,
            tc.tile_pool(name="opool", bufs=4) as opool,
            tc.tile_pool(name="ppool", bufs=6, space="PSUM") as ppool,
            tc.tile_pool(name="dpsum", bufs=2, space="PSUM") as dpsum,
            tc.tile_pool(name="dram", bufs=2, space="DRAM") as dram,
        ):
            gidx = res.tile([P, ntiles * 8], I16)
            nc.sync.dma_start(out=gidx[:], in_=gidx_d[:])
            scmp = res.tile([P, ntiles], BF16)
            nc.sync.dma_start(out=scmp[:], in_=scmp_d[:])
            sew = res.tile([P, ntiles], BF16)
            nc.sync.dma_start(out=sew[:], in_=sew_d[:])
            iota = res.tile([P, P], BF16)
            nc.sync.dma_start(out=iota[:], in_=iota_d[:])
            eyedis = res.tile([P, NB * P], BF16)
            nc.sync.dma_start(out=eyedis[:], in_=eyedis_d[:])
            wcat = res.tile([P, wcat_cols], BF16)
            nc.sync.dma_start(out=wcat[:], in_=wcat_d[:])
            bias = res.tile([P, 8], F32)
            nc.sync.dma_start(out=bias[:], in_=bias_d[:])
            prelua = res.tile([P, 2], F32)
            nc.sync.dma_start(out=prelua[:], in_=prelu_d[:])
            disc = res.tile([P, NB], F32)
            nc.sync.dma_start(out=disc[:], in_=disc_d[:])

            asb_local = {}
            hT = {}

            def dense_block(layer, j, shards):
                pd_ = dpsum.tile([P, HID], F32, tag="dps", name="pd")
                if layer == 0:
                    xbl = xpool.tile([P, FC_IN * P], BF16, tag="xo", name="xo")
                    nc.sync.dma_start(out=xbl[:],
                                      in_=xown_d[j * P:(j + 1) * P, :])
                    for fc in range(FC_IN):
                        nc.tensor.matmul(
                            out=pd_[:], lhsT=xbl[:, fc * P:(fc + 1) * P],
                            rhs=wcat[:, w_off[(0, fc)]:w_off[(0, fc)] + HID],
                            start=(fc == 0), stop=(fc == FC_IN - 1))
                else:
                    gi, bj = j // G, j % G
                    for fc in range(FH):
                        nc.tensor.matmul(
                            out=pd_[:],
                            lhsT=hT[(gi, fc)][:, bj * P:(bj + 1) * P],
                            rhs=wcat[:, w_off[(layer, fc)]:
                                     w_off[(layer, fc)] + HID],
                            start=(fc == 0), stop=(fc == FH - 1))
                asb = akeep.tile([P, HID], BF16, tag=f"asb{j}", name=f"asb{j}")
                asb_local[j] = asb
                nc.scalar.activation(
                    out=asb[:], in_=pd_[:],
                    func=mybir.ActivationFunctionType.Identity,
                    bias=0.0, scale=disc[:, j:j + 1])
                r = _region_of_block(j)
                b0 = (j - RBASE[r]) * P
                nc.sync.dma_start(out=shards[r][b0:b0 + P, :], in_=asb[:])

            def gathers(gi, r, srcs):
                (g, rr, t0, t1) = call_ranges[gi * NPART + r]
                n = t1 - t0
                M = mpool.tile([P, maxg[r] * HID], BF16, tag=f"M{r}",
                               name=f"M{r}")
                ncalls = (n + CT - 1) // CT
                chunk = (n + ncalls - 1) // ncalls
                for k0 in range(0, n, chunk):
                    k1 = min(k0 + chunk, n)
                    nt_ = k1 - k0
                    nc.gpsimd.dma_gather(
                        out_ap=M[:, k0 * HID:k1 * HID].rearrange(
                            "p (t e) -> p t e", e=HID),
                        in_ap=srcs[r],
                        idxs_ap=gidx[:, (t0 + k0) * 8:(t0 + k1) * 8],
                        num_idxs=nt_ * P,
                        num_idxs_reg=nt_ * P,
                        elem_size=HID,
                        queue_num=qctr[0] % 4,
                    )
                    qctr[0] += 1
                S = spool.tile([P, maxg[r] * P], BF16, tag=f"S{r}",
                               name=f"S{r}")
                s3 = S[:, :n * P].rearrange("p (t e) -> p t e", e=P)
                iob = iota[:].rearrange("p (o e) -> p o e", o=1).broadcast_to(
                    [P, n, P])
                nc.vector.tensor_tensor(
                    out=s3, in0=iob, in1=scmp[:, t0:t1].to_broadcast([P, n, P]),
                    op=mybir.AluOpType.is_equal)
                nc.vector.tensor_tensor(
                    out=s3, in0=s3, in1=sew[:, t0:t1].to_broadcast([P, n, P]),
                    op=mybir.AluOpType.mult)
                return M, S, t0

            def mm_part(gi, r, pbf, M, S, t0, blks):
                for bj, j in enumerate(blks):
                    for t in range(tiles[j, r]):
                        tl = tile_base[j, r] - t0 + t
                        last = (r == NPART - 1 and bj == len(blks) - 1
                                and t == tiles[j, r] - 1)
                        for fh in range(FH):
                            nc.tensor.matmul(
                                out=pbf[fh][:, bj * P:(bj + 1) * P],
                                lhsT=M[:, tl * HID + fh * P:
                                       tl * HID + (fh + 1) * P],
                                rhs=S[:, tl * P:(tl + 1) * P],
                                start=False, stop=last)

            def epilogue(layer, gi, pbf, blks):
                gw = len(blks)
                if layer < 3:
                    for fh in range(FH):
                        ht = htpool.tile([P, G * P], BF16,
                                         tag=f"ht{gi}_{fh}", name=f"ht{gi}_{fh}")
                        nc.scalar.activation(
                            out=ht[:, :gw * P], in_=pbf[fh][:, :gw * P],
                            func=mybir.ActivationFunctionType.Identity,
                            bias=bias[:, layer * 2 + fh:layer * 2 + fh + 1],
                            scale=1.0)
                        hT[(gi, fh)] = ht
                else:
                    for fh in range(FH):
                        osb = opool.tile([P, G * P], F32, tag="osb", name="osb")
                        nc.scalar.activation(
                            out=osb[:, :gw * P], in_=pbf[fh][:, :gw * P],
                            func=mybir.ActivationFunctionType.Prelu,
                            bias=bias[:, 6 + fh:7 + fh],
                            scale=1.0, alpha=prelua[:, fh:fh + 1])
                        for bj, j in enumerate(blks):
                            b0 = (fh * NB + j) * P
                            nc.sync.dma_start(
                                out=out_d[b0:b0 + P, :],
                                in_=osb[:, bj * P:(bj + 1) * P])

            def aggregate(layer, srcs, post_group=None):
                gb = group_blocks()
                st = {}
                for s in range(NGRP + 2):
                    if s < NGRP:
                        gi, blks = gb[s]
                        pbf = [ppool.tile([P, G * P], F32, tag="pbf",
                                          name="pbf") for _ in range(FH)]
                        for bj, j in enumerate(blks):
                            for fh in range(FH):
                                nc.tensor.matmul(
                                    out=pbf[fh][:, bj * P:(bj + 1) * P],
                                    lhsT=asb_local[j][:, fh * P:(fh + 1) * P],
                                    rhs=eyedis[:, j * P:(j + 1) * P],
                                    start=(bj == 0), stop=False)
                        M0, S0, t00 = gathers(gi, 0, srcs)
                        st[s] = dict(pbf=pbf, M0=M0, S0=S0, t00=t00)
                    if 0 <= s - 1 < NGRP:
                        gi, blks = gb[s - 1]
                        d = st[s - 1]
                        mm_part(gi, 0, d["pbf"], d["M0"], d["S0"], d["t00"],
                                blks)
                        M1, S1, t01 = gathers(gi, 1, srcs)
                        M2, S2, t02 = gathers(gi, 2, srcs)
                        d.update(M1=M1, S1=S1, t01=t01, M2=M2, S2=S2, t02=t02)
                    if 0 <= s - 2 < NGRP:
                        gi, blks = gb[s - 2]
                        d = st.pop(s - 2)
                        mm_part(gi, 1, d["pbf"], d["M1"], d["S1"], d["t01"],
                                blks)
                        mm_part(gi, 2, d["pbf"], d["M2"], d["S2"], d["t02"],
                                blks)
                        epilogue(layer, gi, d["pbf"], blks)
                        if post_group is not None:
                            post_group(gi)

            # ---------------- layer chain
            def make_fulls():
                return [dram.tile([RSZ[r], HID], BF16, tag=f"f{r}",
                                  name=f"f{r}", addr_space="Shared")
                        for r in range(NPART)]

            def make_shards():
                return [dram.tile([RB[r] * NC * P // NC, HID], BF16,
                                  tag=f"sh{r}", name=f"sh{r}")
                        for r in range(NPART)]

            def emit_ag(shards, fulls, r):
                nc.gpsimd.collective_compute(
                    "AllGather", mybir.AluOpType.bypass,
                    ins=[shards[r][:].opt()], outs=[fulls[r][:].opt()],
                    replica_groups=[list(range(NC))])

            shards1 = make_shards()
            fulls1 = make_fulls()
            for j in range(NB):
                dense_block(0, j, shards1)
                for r in range(NPART):
                    if j == RBASE[r] + RB[r] - 1:
                        emit_ag(shards1, fulls1, r)
            srcs = [f[:, :] for f in fulls1]

            for lw in (1, 2, 3):
                shards = make_shards()
                fulls = make_fulls()

                def post(gi, shards=shards, fulls=fulls, lw=lw):
                    # dense for this group's own blocks right away (spreads
                    # PE/ACT load); AllGather once a region is complete
                    for j in group_blocks()[gi][1]:
                        dense_block(lw, j, shards)
                    for r in range(NPART):
                        if gi == POST_GRP[r]:
                            emit_ag(shards, fulls, r)

                aggregate(lw - 1, srcs, post_group=post)
                srcs = [f[:, :] for f in fulls]

            aggregate(3, srcs)

    nc.compile()
    return nc


# ----------------------------------------------------------------- execution

def _iota_np():
    return np.tile(np.arange(P, dtype=np.float32)[None, :], (P, 1)).astype(
        ml_dtypes.bfloat16)


def _make_in_maps(prep, x, Ws, bs, prelu_a):
    wcat = _pack_wcat(Ws)
    biasp = _pack_bias(bs)
    prelup = np.zeros((P, 2), np.float32)
    prelup[:, 0] = prelu_a[:P]
    prelup[:, 1] = prelu_a[P:]
    iota = _iota_np()
    maps = []
    xf = np.asarray(x, np.float32)
    for c in range(NC):
        maps.append({
            "gidx": prep["gidx"][c],
            "scmp": prep["scmp"][c],
            "sew": prep["sew"][c],
            "iota": iota,
            "eyedis": prep["eyedis"][c],
            "wcat": wcat,
            "bias": biasp,
            "prelua": prelup,
            "disc": prep["disc"][c],
            "xown": _pack_xown(xf, prep["nid"][c]),
        })
    return maps


def _assemble_out(prep, results):
    y = np.zeros((N_NODES, HID), dtype=np.float32)
    nid = prep["nid"]
    for c in range(NC):
        o = results[c]["out"].reshape(FH, NB, P, P)
        for fh in range(FH):
            for j in range(NB):
                nids = nid[c, j]
                ok = nids >= 0
                y[nids[ok], fh * P:(fh + 1) * P] = o[fh, j, :, ok]
    return y


def run(x, edge_index, edge_weight, W1, b1, W2, b2, W3, b3, W4, b4, prelu_a):
    prep = _preprocess(edge_index, edge_weight)
    nc = _build(prep)
    in_maps = _make_in_maps(
        prep, x,
        [np.asarray(W1, np.float32), np.asarray(W2, np.float32),
         np.asarray(W3, np.float32), np.asarray(W4, np.float32)],
        [np.asarray(b1, np.float32), np.asarray(b2, np.float32),
         np.asarray(b3, np.float32), np.asarray(b4, np.float32)],
        np.asarray(prelu_a, np.float32))
    res = run_bass_kernel_spmd(nc, in_maps, core_ids=list(range(NC)))
    return _assemble_out(prep, res.results).astype(np.float32)


def kernel(x, edge_index, edge_weight, W1, b1, W2, b2, W3, b3, W4, b4, prelu_a):
    return run(x, edge_index, edge_weight,
               W1, b1, W2, b2, W3, b3, W4, b4, prelu_a)
